# revision 28
# baseline (speedup 1.0000x reference)
"""Classical self-attention block (QKV proj -> softmax attention -> out proj
-> residual + LayerNorm) on 8 Trainium2 NeuronCores.

Sharding: sequence-parallel over queries. Core c handles batch c//4, query
rows (c%4)*1024 .. +1024. Each core recomputes K/V for its whole batch
(no collectives). The per-batch input is rolled on the host so the core's
query rows are always rows 0..1023 -- softmax over keys is permutation
invariant, so attention output for those queries is unchanged.

v9 (v8 310us was stalled by et-buffer WAW coupling + round boundaries):
  - et ALWAYS fp8e5 (ScalarE LUT Exp / DVE int8-Schraudolph e5m2 bits),
    engine assignment per ITERATION; et bufs=8 so the two exp engines
    never WAW-wait on each other's 3-pairs-old writes.
  - v_t gives every head 64 ones-columns: the PV matmul replicates the
    softmax denominator into pv rows 64..127 for free (PE matmuls are
    rhs-stream-bound). normalize = reciprocal of pv[64:128] + 2 muls
    straight from PSUM -- no pvs copy, no den bounce, no gpsimd
    broadcast. The ones region is seeded by one small memset + 16
    SBUF->SBUF DMAs during the input-DMA window.
  - per-slot emission order: score MM, exp, PV drain, fillers -- the PE
    FIFO serves the exp chain first.
  - LayerNorm emitted in chunk PAIRS (one [128,1024] outproj psum, one
    z-add, one Newton-rstd chain on [128,2]) to shorten the tail.
"""

import numpy as np
import ml_dtypes

import concourse.bass as bass
import concourse.mybir as mybir
import concourse.tile as tile
from concourse import bacc
from concourse.bass_utils import run_bass_kernel_spmd

B, S, D = 2, 4096, 512
H, Dh = 8, 64
SQ = 1024            # query rows per core
SCALE = 1.0 / np.sqrt(Dh)
W8 = 8.0             # fp8 weight pre-scale (keeps e4m3 out of subnormals)
SSC = SCALE / (W8 * W8)   # score descale folded into the exp input scale
LN_EPS = 1e-5
N_CORES = 8

F32 = mybir.dt.float32
BF16 = mybir.dt.bfloat16
I8 = mybir.dt.int8
FP8 = mybir.dt.float8e4   # V values
FP8E = mybir.dt.float8e5  # et weights: huge dynamic range, no inf cliff

# e5m2 Schraudolph: int8 bits = RNE(A8*s_raw + B8) is the e5m2 bit pattern
# of ~exp(s_true) (s_true = SSC*s_raw). Log-domain quantization rms ~5%,
# same order as ScalarE's exp->e5m2 value rounding.
A8 = (4.0 / np.log(2.0)) * SSC
B8 = 60.0 - 0.25          # 4*15 bias; -0.25 centers the bits-vs-value bias

DC = D // 128        # 4 d-chunks (contraction for projections)
EC = D // 128        # 4 e-chunks (output chunks of projections)
TC = S // 128        # 32 t-chunks (keys)
TB = S // 512        # 8 t-blocks of 512
QB = SQ // 512       # 2 query blocks of 512
QC = SQ // 128       # 8 query chunks of 128
NP = TC // 2         # 16 key-chunk pairs per round
VROW = H * 128       # per (pair,sub): 8 heads x [V(64) | ones(64)]

N_DVE_IT = 84        # of 256 exp iterations, how many run on DVE
LAG_IT = 5           # PV for pair p runs after exp of iter 2p+1+LAG_IT-1


def build_assign(n_it, n_dve, guard):
    """Spread n_dve DVE picks, never two consecutive (two slot-chains on
    one engine serialize the pipeline at that engine's latency)."""
    assign = [False] * n_it
    acc, placed = 0.0, 0
    rate = n_dve / n_it
    for g in range(n_it):
        acc = min(acc + rate, 2.0)
        if (acc >= 1.0 and g not in guard and placed < n_dve
                and not (g > 0 and assign[g - 1])):
            assign[g] = True
            acc -= 1.0
            placed += 1
    return assign


def build_nc(reps=1, unit_ln=False):
    nc = bacc.Bacc("TRN2", target_bir_lowering=False, debug=False,
                   num_devices=N_CORES)

    xt = nc.dram_tensor("xt", [128, 2, 2, S], FP8, kind="ExternalInput")
    xq = nc.dram_tensor("xq", [SQ, D], F32, kind="ExternalInput")
    wqt = nc.dram_tensor("wqt", [128, 2, 2, D], FP8, kind="ExternalInput")
    wkt = nc.dram_tensor("wkt", [128, 2, 2, D], FP8, kind="ExternalInput")
    wvt = nc.dram_tensor("wvt", [128, 2, 2, D], FP8, kind="ExternalInput")
    wot = nc.dram_tensor("wot", [D, D], BF16, kind="ExternalInput")
    gamma = nc.dram_tensor("gamma", [D], F32, kind="ExternalInput")
    beta = nc.dram_tensor("beta", [D], F32, kind="ExternalInput")
    out = nc.dram_tensor("out", [SQ, D], F32, kind="ExternalOutput")

    with tile.TileContext(nc) as tc:
        with (
            tc.tile_pool(name="const", bufs=1) as p_const,
            tc.tile_pool(name="wts", bufs=1) as p_w,
            tc.tile_pool(name="xtp", bufs=1) as p_xt,
            tc.tile_pool(name="kt", bufs=1) as p_kt,
            tc.tile_pool(name="vv", bufs=1) as p_v,
            tc.tile_pool(name="qt", bufs=1) as p_qt,
            tc.tile_pool(name="at", bufs=1) as p_at,
            tc.tile_pool(name="xqp", bufs=1) as p_xq,
            tc.tile_pool(name="e8", bufs=8) as p_e8,
            tc.tile_pool(name="nrm", bufs=2) as p_nrm,
            tc.tile_pool(name="ln", bufs=2) as p_ln,
            # PSUM: 3x[128,1024] score/proj/LN slots + [128,1024] pv = 8
            tc.tile_pool(name="psA", bufs=3, space="PSUM") as ps_a,
            tc.tile_pool(name="psPV", bufs=1, space="PSUM") as ps_pv,
        ):
            # ---- act-table warmup (Exp is the only ScalarE function) ----
            warm = p_const.tile([1, 1], F32, tag="warm")
            nc.vector.memset(warm, 1.0)
            nc.scalar.activation(out=warm, in_=warm,
                                 func=mybir.ActivationFunctionType.Exp,
                                 scale=1.0)

            # ---- weights / inputs ----
            w_tiles = {}
            for name, handle in (("wk", wkt), ("wq", wqt), ("wv", wvt)):
                t = p_w.tile([128, 2, 2, D], FP8, tag=name, name=name)
                nc.sync.dma_start(out=t, in_=handle.ap())
                w_tiles[name] = t

            xt_t = p_xt.tile([128, 2, 2, S], FP8, tag="xt8")
            for cc in range(4):
                lo = cc * 1024
                nc.gpsimd.dma_start(
                    out=xt_t[:, :, :, lo:lo + 1024],
                    in_=xt[:, :, :, lo:lo + 1024])
            xq_t = p_xq.tile([128, QC, D], F32, tag="xq")

            wot_t = p_const.tile([128, EC, D], BF16, tag="wo")
            nc.sync.dma_start(
                out=wot_t, in_=wot.ap().rearrange("(c p) e -> p c e", p=128))
            gamma_b = beta_b = None
            if not unit_ln:
                gamma_b = p_const.tile([128, D], F32, tag="gamma_b")
                beta_b = p_const.tile([128, D], F32, tag="beta_b")
                nc.sync.dma_start(
                    out=gamma_b,
                    in_=bass.AP(tensor=gamma, offset=0, ap=[[0, 128], [1, D]]))
                nc.sync.dma_start(
                    out=beta_b,
                    in_=bass.AP(tensor=beta, offset=0, ap=[[0, 128], [1, D]]))

            # ---- persistent activations ----
            kt_t = p_kt.tile([128, EC, S], BF16, tag="kt")       # K^T [e, t]
            qt_t = p_qt.tile([128, EC, SQ], BF16, tag="qt")      # Q^T [e, s]
            # v_t per (pair, sub): 8 heads x [ones(64) | V(64)]. The PV
            # matmul reads 128 lhsT cols per head: rows 0..63 of the output
            # get the denominator REPLICATED (free -- matmuls are stream-
            # bound), rows 64..127 get PV. Ones first: the custom-DVE
            # reciprocal only honors base-0 partition operands.
            v_t = p_v.tile([128, NP, 2, VROW], FP8, tag="v")
            at_t = p_at.tile([128, EC, SQ], BF16, tag="at")      # A^T [e', s]

            # ones template (one full head-column worth) -> copy into every
            # head's ones block with 8 ordinary strided SBUF->SBUF DMAs.
            onet = p_const.tile([128, NP * 2 * 64], FP8, tag="onet")
            nc.vector.memset(onet, 1.0)
            for h in range(H):
                dst = v_t[:, :, :, h * 128:h * 128 + 64].rearrange(
                    "p a b x -> p (a b) x")
                src = onet.rearrange("p (n x) -> p n x", x=64)
                nc.gpsimd.dma_start(out=dst, in_=src)
            # xq only gates LayerNorm -- load it after the ones blocks
            nc.gpsimd.dma_start(
                out=xq_t, in_=xq.ap().rearrange("(n p) e -> p n e", p=128))

            # ---------- emission helpers ----------
            # proj fills are SPLIT: the matmul part runs at its fill slot,
            # the PSUM->SBUF copy is returned and emitted FIRST at the next
            # slot, so it sits at the head of the DVE queue (not behind
            # exps) and the shared sc-slot frees before the chain needs it.
            def emit_kt_pair(ec, k):
                ps = ps_a.tile([128, 1024], F32, tag="sc", name="psk")
                for half in range(2):
                    tb = 2 * k + half
                    for P in range(2):
                        nc.tensor.matmul(
                            ps[:, half * 512:(half + 1) * 512],
                            w_tiles["wk"][:, P, :, ec * 128:(ec + 1) * 128],
                            xt_t[:, P, :, tb * 512:(tb + 1) * 512],
                            start=(P == 0), stop=(P == 1),
                            perf_mode=mybir.MatmulPerfMode.DoubleRow)
                return lambda: nc.vector.tensor_copy(
                    out=kt_t[:, ec, k * 1024:(k + 1) * 1024], in_=ps)

            def emit_qt_full(ec):
                ps = ps_a.tile([128, 1024], F32, tag="sc", name="psq")
                for qb in range(2):
                    for P in range(2):
                        nc.tensor.matmul(
                            ps[:, qb * 512:(qb + 1) * 512],
                            w_tiles["wq"][:, P, :, ec * 128:(ec + 1) * 128],
                            xt_t[:, P, :, qb * 512:(qb + 1) * 512],
                            start=(P == 0), stop=(P == 1),
                            perf_mode=mybir.MatmulPerfMode.DoubleRow)
                return lambda: nc.vector.tensor_copy(
                    out=qt_t[:, ec, :], in_=ps)

            def emit_v_pair(pp):
                ps = ps_a.tile([128, 1024], F32, tag="sc", name="psv")
                for sub in range(2):
                    tcb = 2 * pp + sub
                    for P in range(2):
                        nc.tensor.matmul(
                            ps[:, sub * 512:(sub + 1) * 512],
                            xt_t[:, P, :, tcb * 128:(tcb + 1) * 128],
                            w_tiles["wv"][:, P, :, :],
                            start=(P == 0), stop=(P == 1),
                            perf_mode=mybir.MatmulPerfMode.DoubleRow)
                v_dst = v_t[:, pp, :, :].rearrange(
                    "p b (h x) -> p b h x", x=128)[:, :, :, 64:128]
                return lambda: nc.vector.tensor_copy(
                    out=v_dst,
                    in_=ps.rearrange("p (b h x) -> p b h x", b=2, x=64))

            def emit_normalize(j, qb, pv):
                # rows 0..63 of pv hold the replicated denominator; the
                # muls read PSUM directly (PSUM in0 + SBUF in1 allows
                # mismatched bases; SBUF+SBUF would not -- NCC_IBIR297)
                rc = p_nrm.tile([64, 1024], F32, tag="rc", name="rc")
                nc.vector.reciprocal_approx_fast(out=rc, in_=pv[0:64, :])
                for i in range(2):
                    lo = i * 64
                    nc.vector.tensor_mul(
                        at_t[lo:lo + 64, j, qb * 512:(qb + 1) * 512],
                        pv[64:128, i * 512:(i + 1) * 512],
                        rc[:, i * 512:(i + 1) * 512])

            def emit_ln_pair(s0):
                # phase 1 (at the fill slot): outproj matmuls only.
                # Returns phase 2 (the DVE chain), deferred 2 slots.
                ps = ps_a.tile([128, 1024], F32, tag="sc", name="pso")
                for half in range(2):
                    sc8 = s0 + half
                    for ecp in range(EC):
                        nc.tensor.matmul(
                            ps[:, half * 512:(half + 1) * 512],
                            at_t[:, ecp, sc8 * 128:(sc8 + 1) * 128],
                            wot_t[:, ecp, :],
                            start=(ecp == 0), stop=(ecp == EC - 1))
                return lambda: emit_ln_chain(s0, ps)

            def emit_ln_chain(s0, ps):
                # two 128-row chunks: one z-add, per-chunk bn_stats, ONE
                # Newton-rstd chain on [128,2]
                z = p_ln.tile([128, 2, 512], F32, tag="z", name="z")
                nc.vector.tensor_add(
                    z, ps.rearrange("p (c e) -> p c e", c=2),
                    xq_t[:, s0:s0 + 2, :].rearrange(
                        "p (a c) e -> p (a c e)", a=1).rearrange(
                        "p (c e) -> p c e", c=2))
                stats = p_ln.tile([128, 2, 6], F32, tag="stats", name="stats")
                mv = p_ln.tile([128, 2, 2], F32, tag="mv", name="mv")
                for c in range(2):
                    nc.vector.bn_stats(out=stats[:, c, :], in_=z[:, c, :])
                    nc.vector.bn_aggr(out=mv[:, c, :], in_=stats[:, c, :])
                # rstd = 1/sqrt(var): 3 Newton steps from y0=1, batched [128,2]
                v_ = mv[:, :, 1]
                rstd = p_ln.tile([128, 2], F32, tag="rstd", name="rstd")
                t1 = p_ln.tile([128, 2], F32, tag="t1", name="t1")
                nc.vector.tensor_scalar(
                    out=rstd, in0=v_, scalar1=-0.5, scalar2=1.5,
                    op0=mybir.AluOpType.mult, op1=mybir.AluOpType.add)
                for _ in range(2):
                    nc.vector.tensor_mul(t1, rstd, rstd)
                    nc.vector.tensor_mul(t1, t1, v_)
                    nc.vector.tensor_scalar(
                        out=t1, in0=t1, scalar1=-0.5, scalar2=1.5,
                        op0=mybir.AluOpType.mult, op1=mybir.AluOpType.add)
                    nc.vector.tensor_mul(rstd, rstd, t1)
                for c in range(2):
                    sc8 = s0 + c
                    zc = z[:, c, :]
                    nc.vector.tensor_scalar(
                        out=zc, in0=zc, scalar1=mv[:, c:c + 1, 0].rearrange(
                            "p a -> p a"), scalar2=rstd[:, c:c + 1],
                        op0=mybir.AluOpType.subtract,
                        op1=mybir.AluOpType.mult)
                    if not unit_ln:
                        nc.vector.tensor_mul(zc, zc, gamma_b)
                        nc.vector.tensor_add(zc, zc, beta_b)
                    nc.sync.dma_start(
                        out=out[sc8 * 128:(sc8 + 1) * 128, :], in_=zc)

            def emit_attention_stream():
                rounds = [(qb, j) for qb in range(QB) for j in range(EC)]
                n_it = len(rounds) * TC
                fill = {}
                dve_guard = set()

                def add(g, th, delay=1, heavy=False):
                    # fills return a follow-up thunk that runs at the HEAD
                    # of slot g+delay (ahead of that slot's exp); guard the
                    # landing slot so it isn't queued behind a DVE exp.
                    fill.setdefault(g, []).append((th, delay))
                    dve_guard.add(g + delay)
                    if heavy:
                        dve_guard.update(
                            (g + delay + 1, g + delay + 2, g + delay + 3))

                for pp in range(NP):
                    add(2 * pp, lambda pp=pp: emit_v_pair(pp))
                for k in range(1, 4):
                    add(max(1, 8 * k - 4),
                        lambda k=k: emit_kt_pair(0, k))
                for r in (1, 2, 3):
                    base = (r - 1) * TC + 6
                    add(base, lambda r=r: emit_qt_full(r))
                    for k in range(4):
                        add(base + 2 + 3 * k,
                            lambda r=r, k=k: emit_kt_pair(r, k))
                add(4 * TC + 6, lambda: emit_ln_pair(0), delay=2, heavy=True)
                add(5 * TC + 6, lambda: emit_ln_pair(2), delay=2, heavy=True)

                # round-edge guard: ScalarE owns the edges so the boundary
                # normalize / next-round PV never queue behind DVE exps.
                for r in range(len(rounds)):
                    for e in (0, 1, 2, 3, 30, 31):
                        dve_guard.add(r * TC + e)
                assign = build_assign(n_it, N_DVE_IT, dve_guard)

                pv = ps_pv.tile([128, 1024], F32, tag="pv", name="pv")
                et_tiles = {}
                pend_pv = []

                def emit_pv_pair(p_glob):
                    rp, pp = divmod(p_glob, NP)
                    qb_p, j_p = rounds[rp]
                    et = et_tiles.pop(p_glob)
                    start, stop = (pp == 0), (pp == NP - 1)
                    for i in range(2):
                        h = 2 * j_p + i
                        nc.tensor.matmul(
                            pv[:, i * 512:(i + 1) * 512],
                            v_t[:, pp, :, h * 128:h * 128 + 128],
                            et[:, :, i * 512:(i + 1) * 512],
                            start=start, stop=stop,
                            perf_mode=mybir.MatmulPerfMode.DoubleRow)
                    if stop:
                        emit_normalize(j_p, qb_p, pv)

                deferred = {}
                for g in range(n_it + LAG_IT + 6):
                    for cp in deferred.pop(g, ()):
                        cp()
                    if g < n_it:
                        r = g // TC
                        qb, j = rounds[r]
                        u = g % TC
                        p_glob = r * NP + u // 2
                        sc = ps_a.tile([128, 1024], F32, tag="sc", name="sc")
                        for i in range(2):
                            lo = i * 64
                            nc.tensor.matmul(
                                sc[:, i * 512:(i + 1) * 512],
                                kt_t[lo:lo + 64, j, u * 128:(u + 1) * 128],
                                qt_t[lo:lo + 64, j,
                                     qb * 512:(qb + 1) * 512],
                                start=True, stop=True,
                                tile_position=(lo, 0))
                        if u % 2 == 0:
                            et = p_e8.tile([128, 2, 1024], FP8E, tag="e8",
                                           name="e8")
                            et_tiles[p_glob] = et
                        else:
                            et = et_tiles[p_glob]
                        if assign[g]:
                            nc.vector.tensor_scalar(
                                out=et[:, u % 2, :].bitcast(I8), in0=sc,
                                scalar1=A8, scalar2=B8,
                                op0=mybir.AluOpType.mult,
                                op1=mybir.AluOpType.add)
                        else:
                            nc.scalar.activation(
                                out=et[:, u % 2, :], in_=sc,
                                func=mybir.ActivationFunctionType.Exp,
                                scale=SSC)
                        if u % 2 == 1:
                            # first pair of a round lags extra (waits the
                            # previous round's normalize to free pv); the
                            # last pair hurries so normalize starts early
                            if u // 2 == 0:
                                lag = LAG_IT + 3
                            elif u // 2 == NP - 1:
                                lag = LAG_IT - 2
                            else:
                                lag = LAG_IT
                            pend_pv.append((g + lag, p_glob))
                    while pend_pv and pend_pv[0][0] <= g:
                        emit_pv_pair(pend_pv.pop(0)[1])
                    for th, delay in fill.pop(g, ()):
                        cp = th()
                        if cp is not None:
                            deferred.setdefault(g + delay, []).append(cp)
                for cps in deferred.values():
                    for cp in cps:
                        cp()

            # ---------- program order ----------
            for _rep in range(reps):
                cp1 = emit_kt_pair(0, 0)
                cp2 = emit_qt_full(0)
                cp1()
                cp2()
                emit_attention_stream()
                ch1 = emit_ln_pair(4)
                ch2 = emit_ln_pair(6)
                ch1()
                ch2()

    nc.finalize()
    return nc


_NC = None
_NC_KIND = None


def kernel(rotation_params=None, entangle_params=None, inputs=None,
           Wq=None, Wk=None, Wv=None, Wo=None, ln_gamma=None, ln_beta=None,
           _trace=False, **_unused):
    global _NC
    X = np.ascontiguousarray(np.asarray(inputs, np.float32))
    Wq = np.asarray(Wq, np.float32)
    Wk = np.asarray(Wk, np.float32)
    Wv = np.asarray(Wv, np.float32)
    Wo = np.asarray(Wo, np.float32)
    g = np.ascontiguousarray(np.asarray(ln_gamma, np.float32))
    b = np.ascontiguousarray(np.asarray(ln_beta, np.float32))

    def dr8(M):  # [D, N] -> [128, P2, k2, N] fp8 DoubleRow layout
        return np.ascontiguousarray(
            M.reshape(2, 2, 128, -1).transpose(2, 0, 1, 3)
        ).astype(ml_dtypes.float8_e4m3)

    wqt = dr8(Wq.T * W8)
    wkt = dr8(Wk.T * W8)
    wvt = dr8(Wv.T * W8)
    wot = np.ascontiguousarray(Wo.T / W8).astype(ml_dtypes.bfloat16)

    in_maps = []
    for c in range(N_CORES):
        bb, q0 = c // 4, (c % 4) * SQ
        Xb = np.roll(X[bb], -q0, axis=0)
        in_maps.append({
            "xt": dr8(Xb.T),
            "xq": np.ascontiguousarray(Xb[:SQ]),
            "wqt": wqt, "wkt": wkt, "wvt": wvt, "wot": wot,
            "gamma": g, "beta": b,
        })

    unit_ln = bool(np.all(g == 1.0) and np.all(b == 0.0))
    global _NC_KIND
    if _NC is None or _NC_KIND != unit_ln:
        _NC = build_nc(unit_ln=unit_ln)
        _NC_KIND = unit_ln
    res = run_bass_kernel_spmd(_NC, in_maps, core_ids=list(range(N_CORES)),
                               trace=_trace)
    out = np.empty((B, S, D), np.float32)
    for c in range(N_CORES):
        bb, q0 = c // 4, (c % 4) * SQ
        out[bb, q0:q0 + SQ] = res.results[c]["out"]
    if _trace:
        kernel._last_results = res
    return out


# revision 31
# speedup vs baseline: 1.0221x; 1.0221x over previous
"""Classical self-attention block (QKV proj -> softmax attention -> out proj
-> residual + LayerNorm) on 8 Trainium2 NeuronCores.

Sharding: sequence-parallel over queries. Core c handles batch c//4, query
rows (c%4)*1024 .. +1024. Each core recomputes K/V for its whole batch
(no collectives). The per-batch input is rolled on the host so the core's
query rows are always rows 0..1023 -- softmax over keys is permutation
invariant, so attention output for those queries is unchanged.

v9 (v8 310us was stalled by et-buffer WAW coupling + round boundaries):
  - et ALWAYS fp8e5 (ScalarE LUT Exp / DVE int8-Schraudolph e5m2 bits),
    engine assignment per ITERATION; et bufs=8 so the two exp engines
    never WAW-wait on each other's 3-pairs-old writes.
  - v_t gives every head 64 ones-columns: the PV matmul replicates the
    softmax denominator into pv rows 64..127 for free (PE matmuls are
    rhs-stream-bound). normalize = reciprocal of pv[64:128] + 2 muls
    straight from PSUM -- no pvs copy, no den bounce, no gpsimd
    broadcast. The ones region is seeded by one small memset + 16
    SBUF->SBUF DMAs during the input-DMA window.
  - per-slot emission order: score MM, exp, PV drain, fillers -- the PE
    FIFO serves the exp chain first.
  - LayerNorm emitted in chunk PAIRS (one [128,1024] outproj psum, one
    z-add, one Newton-rstd chain on [128,2]) to shorten the tail.
"""

import numpy as np
import ml_dtypes

import concourse.bass as bass
import concourse.mybir as mybir
import concourse.tile as tile
from concourse import bacc
from concourse.bass_utils import run_bass_kernel_spmd

B, S, D = 2, 4096, 512
H, Dh = 8, 64
SQ = 1024            # query rows per core
SCALE = 1.0 / np.sqrt(Dh)
W8 = 8.0             # fp8 weight pre-scale (keeps e4m3 out of subnormals)
SSC = SCALE / (W8 * W8)   # score descale folded into the exp input scale
LN_EPS = 1e-5
N_CORES = 8

F32 = mybir.dt.float32
BF16 = mybir.dt.bfloat16
I8 = mybir.dt.int8
FP8 = mybir.dt.float8e4   # V values
FP8E = mybir.dt.float8e5  # et weights: huge dynamic range, no inf cliff

# e5m2 Schraudolph: int8 bits = RNE(A8*s_raw + B8) is the e5m2 bit pattern
# of ~exp(s_true) (s_true = SSC*s_raw). Log-domain quantization rms ~5%,
# same order as ScalarE's exp->e5m2 value rounding.
A8 = (4.0 / np.log(2.0)) * SSC
B8 = 60.0 - 0.25          # 4*15 bias; -0.25 centers the bits-vs-value bias

DC = D // 128        # 4 d-chunks (contraction for projections)
EC = D // 128        # 4 e-chunks (output chunks of projections)
TC = S // 128        # 32 t-chunks (keys)
TB = S // 512        # 8 t-blocks of 512
QB = SQ // 512       # 2 query blocks of 512
QC = SQ // 128       # 8 query chunks of 128
NP = TC // 2         # 16 key-chunk pairs per round
VROW = H * 128       # per (pair,sub): 8 heads x [V(64) | ones(64)]

N_DVE_IT = 84        # of 256 exp iterations, how many run on DVE
LAG_IT = 5           # PV for pair p runs after exp of iter 2p+1+LAG_IT-1


def build_assign(n_it, n_dve, guard):
    """Spread n_dve DVE picks, never two consecutive (two slot-chains on
    one engine serialize the pipeline at that engine's latency)."""
    assign = [False] * n_it
    acc, placed = 0.0, 0
    rate = n_dve / n_it
    for g in range(n_it):
        acc = min(acc + rate, 2.0)
        if (acc >= 1.0 and g not in guard and placed < n_dve
                and not (g > 0 and assign[g - 1])):
            assign[g] = True
            acc -= 1.0
            placed += 1
    return assign


def build_nc(reps=1, unit_ln=False):
    nc = bacc.Bacc("TRN2", target_bir_lowering=False, debug=False,
                   num_devices=N_CORES)

    xt = nc.dram_tensor("xt", [128, 2, 2, S], FP8, kind="ExternalInput")
    xq = nc.dram_tensor("xq", [SQ, D], F32, kind="ExternalInput")
    wqt = nc.dram_tensor("wqt", [128, 2, 2, D], FP8, kind="ExternalInput")
    wkt = nc.dram_tensor("wkt", [128, 2, 2, D], FP8, kind="ExternalInput")
    wvt = nc.dram_tensor("wvt", [128, 2, 2, D], FP8, kind="ExternalInput")
    wot = nc.dram_tensor("wot", [D, D], BF16, kind="ExternalInput")
    gamma = nc.dram_tensor("gamma", [D], F32, kind="ExternalInput")
    beta = nc.dram_tensor("beta", [D], F32, kind="ExternalInput")
    out = nc.dram_tensor("out", [SQ, D], F32, kind="ExternalOutput")

    with tile.TileContext(nc) as tc:
        with (
            tc.tile_pool(name="const", bufs=1) as p_const,
            tc.tile_pool(name="wts", bufs=1) as p_w,
            tc.tile_pool(name="xtp", bufs=1) as p_xt,
            tc.tile_pool(name="kt", bufs=1) as p_kt,
            tc.tile_pool(name="vv", bufs=1) as p_v,
            tc.tile_pool(name="qt", bufs=1) as p_qt,
            tc.tile_pool(name="at", bufs=1) as p_at,
            tc.tile_pool(name="xqp", bufs=1) as p_xq,
            tc.tile_pool(name="e8", bufs=8) as p_e8,
            tc.tile_pool(name="nrm", bufs=2) as p_nrm,
            tc.tile_pool(name="ln", bufs=2) as p_ln,
            # PSUM: 3x[128,1024] score/proj/LN slots + [128,1024] pv = 8
            tc.tile_pool(name="psA", bufs=3, space="PSUM") as ps_a,
            tc.tile_pool(name="psPV", bufs=1, space="PSUM") as ps_pv,
        ):
            # ---- act-table warmup (Exp is the only ScalarE function) ----
            warm = p_const.tile([1, 1], F32, tag="warm")
            nc.vector.memset(warm, 1.0)
            nc.scalar.activation(out=warm, in_=warm,
                                 func=mybir.ActivationFunctionType.Exp,
                                 scale=1.0)

            # ---- weights / inputs ----
            w_tiles = {}
            for name, handle in (("wk", wkt), ("wq", wqt), ("wv", wvt)):
                t = p_w.tile([128, 2, 2, D], FP8, tag=name, name=name)
                nc.sync.dma_start(out=t, in_=handle.ap())
                w_tiles[name] = t

            xt_t = p_xt.tile([128, 2, 2, S], FP8, tag="xt8")
            for cc in range(4):
                lo = cc * 1024
                nc.gpsimd.dma_start(
                    out=xt_t[:, :, :, lo:lo + 1024],
                    in_=xt[:, :, :, lo:lo + 1024])
            xq_t = p_xq.tile([128, QC, D], F32, tag="xq")

            wot_t = p_const.tile([128, EC, D], BF16, tag="wo")
            nc.sync.dma_start(
                out=wot_t, in_=wot.ap().rearrange("(c p) e -> p c e", p=128))
            gamma_b = beta_b = None
            if not unit_ln:
                gamma_b = p_const.tile([128, D], F32, tag="gamma_b")
                beta_b = p_const.tile([128, D], F32, tag="beta_b")
                nc.sync.dma_start(
                    out=gamma_b,
                    in_=bass.AP(tensor=gamma, offset=0, ap=[[0, 128], [1, D]]))
                nc.sync.dma_start(
                    out=beta_b,
                    in_=bass.AP(tensor=beta, offset=0, ap=[[0, 128], [1, D]]))

            # ---- persistent activations ----
            kt_t = p_kt.tile([128, EC, S], BF16, tag="kt")       # K^T [e, t]
            qt_t = p_qt.tile([128, EC, SQ], BF16, tag="qt")      # Q^T [e, s]
            # v_t per (pair, sub): 8 heads x [ones(64) | V(64)]. The PV
            # matmul reads 128 lhsT cols per head: rows 0..63 of the output
            # get the denominator REPLICATED (free -- matmuls are stream-
            # bound), rows 64..127 get PV. Ones first: the custom-DVE
            # reciprocal only honors base-0 partition operands.
            v_t = p_v.tile([128, NP, 2, VROW], FP8, tag="v")
            at_t = p_at.tile([128, EC, SQ], BF16, tag="at")      # A^T [e', s]

            # ones template (one full head-column worth) -> copy into every
            # head's ones block with 8 ordinary strided SBUF->SBUF DMAs.
            onet = p_const.tile([128, NP * 2 * 64], FP8, tag="onet")
            nc.vector.memset(onet, 1.0)
            for h in range(H):
                dst = v_t[:, :, :, h * 128:h * 128 + 64].rearrange(
                    "p a b x -> p (a b) x")
                src = onet.rearrange("p (n x) -> p n x", x=64)
                nc.gpsimd.dma_start(out=dst, in_=src)
            # xq only gates LayerNorm -- load it after the ones blocks
            nc.gpsimd.dma_start(
                out=xq_t, in_=xq.ap().rearrange("(n p) e -> p n e", p=128))

            # ---------- emission helpers ----------
            # proj fills are SPLIT: the matmul part runs at its fill slot,
            # the PSUM->SBUF copy is returned and emitted FIRST at the next
            # slot, so it sits at the head of the DVE queue (not behind
            # exps) and the shared sc-slot frees before the chain needs it.
            def emit_kt_pair(ec, k):
                ps = ps_a.tile([128, 1024], F32, tag="sc", name="psk")
                for half in range(2):
                    tb = 2 * k + half
                    for P in range(2):
                        nc.tensor.matmul(
                            ps[:, half * 512:(half + 1) * 512],
                            w_tiles["wk"][:, P, :, ec * 128:(ec + 1) * 128],
                            xt_t[:, P, :, tb * 512:(tb + 1) * 512],
                            start=(P == 0), stop=(P == 1),
                            perf_mode=mybir.MatmulPerfMode.DoubleRow)
                return lambda: nc.vector.tensor_copy(
                    out=kt_t[:, ec, k * 1024:(k + 1) * 1024], in_=ps)

            def emit_kt_half(ec, tb, scalar_copy=False):
                ps = ps_a.tile([128, 512], F32, tag="sc", name="pskh")
                for P in range(2):
                    nc.tensor.matmul(
                        ps,
                        w_tiles["wk"][:, P, :, ec * 128:(ec + 1) * 128],
                        xt_t[:, P, :, tb * 512:(tb + 1) * 512],
                        start=(P == 0), stop=(P == 1),
                        perf_mode=mybir.MatmulPerfMode.DoubleRow)
                dst = kt_t[:, ec, tb * 512:(tb + 1) * 512]
                if scalar_copy:
                    return lambda: nc.scalar.copy(out=dst, in_=ps)
                return lambda: nc.vector.tensor_copy(out=dst, in_=ps)

            def emit_qt_half(ec, qb, scalar_copy=False):
                ps = ps_a.tile([128, 512], F32, tag="sc", name="psqh")
                for P in range(2):
                    nc.tensor.matmul(
                        ps,
                        w_tiles["wq"][:, P, :, ec * 128:(ec + 1) * 128],
                        xt_t[:, P, :, qb * 512:(qb + 1) * 512],
                        start=(P == 0), stop=(P == 1),
                        perf_mode=mybir.MatmulPerfMode.DoubleRow)
                dst = qt_t[:, ec, qb * 512:(qb + 1) * 512]
                if scalar_copy:
                    return lambda: nc.scalar.copy(out=dst, in_=ps)
                return lambda: nc.vector.tensor_copy(out=dst, in_=ps)

            def emit_v_pair(pp):
                ps = ps_a.tile([128, 1024], F32, tag="sc", name="psv")
                for sub in range(2):
                    tcb = 2 * pp + sub
                    for P in range(2):
                        nc.tensor.matmul(
                            ps[:, sub * 512:(sub + 1) * 512],
                            xt_t[:, P, :, tcb * 128:(tcb + 1) * 128],
                            w_tiles["wv"][:, P, :, :],
                            start=(P == 0), stop=(P == 1),
                            perf_mode=mybir.MatmulPerfMode.DoubleRow)
                v_dst = v_t[:, pp, :, :].rearrange(
                    "p b (h x) -> p b h x", x=128)[:, :, :, 64:128]
                return lambda: nc.vector.tensor_copy(
                    out=v_dst,
                    in_=ps.rearrange("p (b h x) -> p b h x", b=2, x=64))

            def emit_normalize(j, qb, pv):
                # rows 0..63 of pv hold the replicated denominator; the
                # muls read PSUM directly (PSUM in0 + SBUF in1 allows
                # mismatched bases; SBUF+SBUF would not -- NCC_IBIR297)
                rc = p_nrm.tile([64, 1024], F32, tag="rc", name="rc")
                nc.vector.reciprocal_approx_fast(out=rc, in_=pv[0:64, :])
                for i in range(2):
                    lo = i * 64
                    nc.vector.tensor_mul(
                        at_t[lo:lo + 64, j, qb * 512:(qb + 1) * 512],
                        pv[64:128, i * 512:(i + 1) * 512],
                        rc[:, i * 512:(i + 1) * 512])

            def emit_ln_pair(s0):
                # phase 1 (at the fill slot): outproj matmuls only.
                # Returns phase 2 (the DVE chain), deferred 2 slots.
                ps = ps_a.tile([128, 1024], F32, tag="sc", name="pso")
                for half in range(2):
                    sc8 = s0 + half
                    for ecp in range(EC):
                        nc.tensor.matmul(
                            ps[:, half * 512:(half + 1) * 512],
                            at_t[:, ecp, sc8 * 128:(sc8 + 1) * 128],
                            wot_t[:, ecp, :],
                            start=(ecp == 0), stop=(ecp == EC - 1))
                return lambda: emit_ln_chain(s0, ps)

            def emit_ln_chain(s0, ps):
                # two 128-row chunks: one z-add, per-chunk bn_stats, ONE
                # Newton-rstd chain on [128,2]
                z = p_ln.tile([128, 2, 512], F32, tag="z", name="z")
                nc.vector.tensor_add(
                    z, ps.rearrange("p (c e) -> p c e", c=2),
                    xq_t[:, s0:s0 + 2, :].rearrange(
                        "p (a c) e -> p (a c e)", a=1).rearrange(
                        "p (c e) -> p c e", c=2))
                stats = p_ln.tile([128, 2, 6], F32, tag="stats", name="stats")
                mv = p_ln.tile([128, 2, 2], F32, tag="mv", name="mv")
                for c in range(2):
                    nc.vector.bn_stats(out=stats[:, c, :], in_=z[:, c, :])
                    nc.vector.bn_aggr(out=mv[:, c, :], in_=stats[:, c, :])
                # rstd = 1/sqrt(var): 3 Newton steps from y0=1, batched [128,2]
                v_ = mv[:, :, 1]
                rstd = p_ln.tile([128, 2], F32, tag="rstd", name="rstd")
                t1 = p_ln.tile([128, 2], F32, tag="t1", name="t1")
                nc.vector.tensor_scalar(
                    out=rstd, in0=v_, scalar1=-0.5, scalar2=1.5,
                    op0=mybir.AluOpType.mult, op1=mybir.AluOpType.add)
                for _ in range(2):
                    nc.vector.tensor_mul(t1, rstd, rstd)
                    nc.vector.tensor_mul(t1, t1, v_)
                    nc.vector.tensor_scalar(
                        out=t1, in0=t1, scalar1=-0.5, scalar2=1.5,
                        op0=mybir.AluOpType.mult, op1=mybir.AluOpType.add)
                    nc.vector.tensor_mul(rstd, rstd, t1)
                for c in range(2):
                    sc8 = s0 + c
                    zc = z[:, c, :]
                    nc.vector.tensor_scalar(
                        out=zc, in0=zc, scalar1=mv[:, c:c + 1, 0].rearrange(
                            "p a -> p a"), scalar2=rstd[:, c:c + 1],
                        op0=mybir.AluOpType.subtract,
                        op1=mybir.AluOpType.mult)
                    if not unit_ln:
                        nc.vector.tensor_mul(zc, zc, gamma_b)
                        nc.vector.tensor_add(zc, zc, beta_b)
                    nc.sync.dma_start(
                        out=out[sc8 * 128:(sc8 + 1) * 128, :], in_=zc)

            def emit_attention_stream():
                rounds = [(qb, j) for qb in range(QB) for j in range(EC)]
                n_it = len(rounds) * TC
                fill = {}
                dve_guard = set()

                def add(g, th, delay=1, heavy=False):
                    # fills return a follow-up thunk that runs at the HEAD
                    # of slot g+delay (ahead of that slot's exp); guard the
                    # landing slot so it isn't queued behind a DVE exp.
                    fill.setdefault(g, []).append((th, delay))
                    dve_guard.add(g + delay)
                    if heavy:
                        dve_guard.update(
                            (g + delay + 1, g + delay + 2, g + delay + 3))

                # v pairs 0..3 were emitted in the prologue; the rest are
                # spaced 3 slots (pair pp gates PV at ~2pp+1+LAG_IT)
                add(0, lambda: emit_kt_half(0, 1))
                for k, pp in enumerate(range(4, NP)):
                    add(1 + 3 * k, lambda pp=pp: emit_v_pair(pp))
                for k in (1, 2, 3):
                    add(8 * k - 3, lambda k=k: emit_kt_pair(0, k))
                for r in (1, 2, 3):
                    base = (r - 1) * TC + 6
                    add(base, lambda r=r: emit_qt_half(r, 0))
                    for k in range(4):
                        add(base + 2 + 3 * k,
                            lambda r=r, k=k: emit_kt_pair(r, k))
                # qb1 q-projections are first used in rounds 4..7
                for jq in range(4):
                    add((3 + jq) * TC + 16,
                        lambda jq=jq: emit_qt_half(jq, 1))
                add(4 * TC + 6, lambda: emit_ln_pair(0), delay=2, heavy=True)
                add(5 * TC + 6, lambda: emit_ln_pair(2), delay=2, heavy=True)

                # round-edge guard: ScalarE owns the edges so the boundary
                # normalize / next-round PV never queue behind DVE exps.
                for r in range(len(rounds)):
                    for e in (0, 1, 2, 3, 30, 31):
                        dve_guard.add(r * TC + e)
                assign = build_assign(n_it, N_DVE_IT, dve_guard)

                pv = ps_pv.tile([128, 1024], F32, tag="pv", name="pv")
                et_tiles = {}
                pend_pv = []

                def emit_pv_pair(p_glob):
                    rp, pp = divmod(p_glob, NP)
                    qb_p, j_p = rounds[rp]
                    et = et_tiles.pop(p_glob)
                    start, stop = (pp == 0), (pp == NP - 1)
                    for i in range(2):
                        h = 2 * j_p + i
                        nc.tensor.matmul(
                            pv[:, i * 512:(i + 1) * 512],
                            v_t[:, pp, :, h * 128:h * 128 + 128],
                            et[:, :, i * 512:(i + 1) * 512],
                            start=start, stop=stop,
                            perf_mode=mybir.MatmulPerfMode.DoubleRow)
                    if stop:
                        emit_normalize(j_p, qb_p, pv)

                deferred = {}
                for g in range(n_it + LAG_IT + 6):
                    for cp in deferred.pop(g, ()):
                        cp()
                    if g < n_it:
                        r = g // TC
                        qb, j = rounds[r]
                        u = g % TC
                        p_glob = r * NP + u // 2
                        sc = ps_a.tile([128, 1024], F32, tag="sc", name="sc")
                        for i in range(2):
                            lo = i * 64
                            nc.tensor.matmul(
                                sc[:, i * 512:(i + 1) * 512],
                                kt_t[lo:lo + 64, j, u * 128:(u + 1) * 128],
                                qt_t[lo:lo + 64, j,
                                     qb * 512:(qb + 1) * 512],
                                start=True, stop=True,
                                tile_position=(lo, 0))
                        if u % 2 == 0:
                            et = p_e8.tile([128, 2, 1024], FP8E, tag="e8",
                                           name="e8")
                            et_tiles[p_glob] = et
                        else:
                            et = et_tiles[p_glob]
                        if assign[g]:
                            nc.vector.tensor_scalar(
                                out=et[:, u % 2, :].bitcast(I8), in0=sc,
                                scalar1=A8, scalar2=B8,
                                op0=mybir.AluOpType.mult,
                                op1=mybir.AluOpType.add)
                        else:
                            nc.scalar.activation(
                                out=et[:, u % 2, :], in_=sc,
                                func=mybir.ActivationFunctionType.Exp,
                                scale=SSC)
                        if u % 2 == 1:
                            # first pair of a round lags extra (waits the
                            # previous round's normalize to free pv); the
                            # last pair hurries so normalize starts early
                            if u // 2 == 0:
                                lag = LAG_IT + 3
                            elif u // 2 == NP - 1:
                                lag = LAG_IT - 2
                            else:
                                lag = LAG_IT
                            pend_pv.append((g + lag, p_glob))
                    while pend_pv and pend_pv[0][0] <= g:
                        emit_pv_pair(pend_pv.pop(0)[1])
                    for th, delay in fill.pop(g, ()):
                        cp = th()
                        if cp is not None:
                            deferred.setdefault(g + delay, []).append(cp)
                for cps in deferred.values():
                    for cp in cps:
                        cp()

            # ---------- program order ----------
            for _rep in range(reps):
                # minimal prologue: only what score-iteration 0 needs (kt
                # t-block 0, qt q-block 0), copies split across DVE and
                # ScalarE; then the first V projections ride the ramp.
                cp1 = emit_kt_half(0, 0)
                cp2 = emit_qt_half(0, 0, scalar_copy=True)
                cp1()
                cp2()
                vcps = [emit_v_pair(pp) for pp in range(2)]
                for cp in vcps:
                    cp()
                vcps = [emit_v_pair(pp) for pp in range(2, 4)]
                for cp in vcps:
                    cp()
                emit_attention_stream()
                ch1 = emit_ln_pair(4)
                ch2 = emit_ln_pair(6)
                ch1()
                ch2()

    nc.finalize()
    return nc


_NC = None
_NC_KIND = None


def kernel(rotation_params=None, entangle_params=None, inputs=None,
           Wq=None, Wk=None, Wv=None, Wo=None, ln_gamma=None, ln_beta=None,
           _trace=False, **_unused):
    global _NC
    X = np.ascontiguousarray(np.asarray(inputs, np.float32))
    Wq = np.asarray(Wq, np.float32)
    Wk = np.asarray(Wk, np.float32)
    Wv = np.asarray(Wv, np.float32)
    Wo = np.asarray(Wo, np.float32)
    g = np.ascontiguousarray(np.asarray(ln_gamma, np.float32))
    b = np.ascontiguousarray(np.asarray(ln_beta, np.float32))

    def dr8(M):  # [D, N] -> [128, P2, k2, N] fp8 DoubleRow layout
        return np.ascontiguousarray(
            M.reshape(2, 2, 128, -1).transpose(2, 0, 1, 3)
        ).astype(ml_dtypes.float8_e4m3)

    wqt = dr8(Wq.T * W8)
    wkt = dr8(Wk.T * W8)
    wvt = dr8(Wv.T * W8)
    wot = np.ascontiguousarray(Wo.T / W8).astype(ml_dtypes.bfloat16)

    in_maps = []
    for c in range(N_CORES):
        bb, q0 = c // 4, (c % 4) * SQ
        Xb = np.roll(X[bb], -q0, axis=0)
        in_maps.append({
            "xt": dr8(Xb.T),
            "xq": np.ascontiguousarray(Xb[:SQ]),
            "wqt": wqt, "wkt": wkt, "wvt": wvt, "wot": wot,
            "gamma": g, "beta": b,
        })

    unit_ln = bool(np.all(g == 1.0) and np.all(b == 0.0))
    global _NC_KIND
    if _NC is None or _NC_KIND != unit_ln:
        _NC = build_nc(unit_ln=unit_ln)
        _NC_KIND = unit_ln
    res = run_bass_kernel_spmd(_NC, in_maps, core_ids=list(range(N_CORES)),
                               trace=_trace)
    out = np.empty((B, S, D), np.float32)
    for c in range(N_CORES):
        bb, q0 = c // 4, (c % 4) * SQ
        out[bb, q0:q0 + SQ] = res.results[c]["out"]
    if _trace:
        kernel._last_results = res
    return out
